# revision 28
# baseline (speedup 1.0000x reference)
"""Multi-head attention (B=2, S=2048, D=1024, H=16, d_k=64) on 8 TRN2 NeuronCores.

Sharding: head-parallel. Core c owns heads (2c, 2c+1) for both batch rows:
 - replicated inputs: qT/kT/vT = x.reshape(B*S, D).T  in bf16, [1024, 4096]
   (D on partitions so the TensorEngine contracts over D with no transposes)
 - per-core weights: Wq columns / Wo rows for its two heads
 - per-core output: partial = attn_out(own heads) @ Wo[own rows]  [4096, 1024] bf16
   The host sums the 8 partials (f32) and adds bo.  No cross-core comm.

Per-core dataflow (bf16 matmuls, f32 PSUM):
 1. qhT/khT [128, 2048] per batch = Wq_c.T @ xT (+bq), d-major accumulation.
    vh [2048, 130] natural = vT.T @ Wqv_c; Wqv has zero-cols / bqv has
    1.0-cols so each head gets a ones column -> attn@V also produces the
    softmax denominators.
 2. scoresT[t,s] = khT.T @ qhT, both heads packed into disjoint PE
    row-groups (K=64).  exp(x/8) on ScalarE from PSUM, bf16 out.
 3. attn@V accumulated over t; row 64 = denominator.  Normalize via
    f32r denominator broadcast (K=1 matmul) + fast reciprocal + mul.
 4. partial[s, :] = outT.T @ Wo_c -> bf16 -> DRAM.

Scheduling: ScalarE (exp, ~142us) and the TensorEngine (~150us) are both
near-saturated, so emission order keeps ACT fed: each s-chunk's
scores+exp loop is emitted first; its attn@V/normalize/out-proj are
deferred one s-chunk and re-emitted between later score iterations via a
two-lane work queue (lane A: DMA-gated b1 projection work with
earliest-iteration thresholds; lane B: always-ready deferred attention).
"""

import numpy as np
import ml_dtypes

B, S, D, H, DK = 2, 2048, 1024, 16, 64
NCORES = 8
HPC = H // NCORES          # heads per core = 2
BS = B * S                 # 4096
HD = HPC * DK              # 128 = per-core head dims

_cache = {}


def _build():
    import concourse.bass as bass
    import concourse.tile as tile
    from concourse import bacc, mybir

    f32 = mybir.dt.float32
    f32r = mybir.dt.float32r
    bf16 = mybir.dt.bfloat16
    Exp = mybir.ActivationFunctionType.Exp

    nc = bacc.Bacc("TRN2", target_bir_lowering=False, debug=False,
                   num_devices=NCORES)

    qT = nc.declare_dram_parameter("qT", [D, BS], bf16, isOutput=False)
    kT = nc.declare_dram_parameter("kT", [D, BS], bf16, isOutput=False)
    vT = nc.declare_dram_parameter("vT", [D, BS], bf16, isOutput=False)
    wq = nc.declare_dram_parameter("wq", [D, HD], bf16, isOutput=False)
    wqv = nc.declare_dram_parameter("wqv", [D, 130], bf16, isOutput=False)
    bq = nc.declare_dram_parameter("bq", [1, HD], f32, isOutput=False)
    bqv = nc.declare_dram_parameter("bqv", [1, 130], f32, isOutput=False)
    wo = nc.declare_dram_parameter("wo", [HD, D], bf16, isOutput=False)
    out = nc.declare_dram_parameter("out", [BS, D], bf16, isOutput=True)

    ND = D // 128            # 8 d-chunks
    NT = S // 128            # 16 t-chunks per batch
    NSC = S // 512           # 4 s-chunks per batch

    with tile.TileContext(nc) as tc:
        with (
            tc.tile_pool(name="const", bufs=1) as pc,
            tc.tile_pool(name="xin", bufs=24) as pin,
            tc.tile_pool(name="proj", bufs=2) as pproj,
            tc.tile_pool(name="vh", bufs=2) as pvh,
            tc.tile_pool(name="exp", bufs=20) as pexp,
            tc.tile_pool(name="outT", bufs=2) as poutT,
            tc.tile_pool(name="small", bufs=2) as psmall,
            tc.tile_pool(name="ob", bufs=4) as pob,
            tc.tile_pool(name="ps", bufs=2, space="PSUM") as pps,
        ):
            # ---- constants (gpsimd queue; bq first — PE needs it early) ----
            bq_row = pc.tile([1, HD], f32)
            nc.gpsimd.dma_start(bq_row[:], bq[:, :])
            bqv_row = pc.tile([1, 130], f32)
            nc.gpsimd.dma_start(bqv_row[:], bqv[:, :])
            wq_sb = pc.tile([128, ND, HD], bf16)
            for d in range(ND):
                nc.gpsimd.dma_start(wq_sb[:, d, :], wq[d * 128:(d + 1) * 128, :])
            wqv_sb = pc.tile([128, ND, 130], bf16)
            for d in range(ND):
                nc.gpsimd.dma_start(wqv_sb[:, d, :], wqv[d * 128:(d + 1) * 128, :])
            wo_sb = pc.tile([HD, D], bf16)
            nc.gpsimd.dma_start(wo_sb[:], wo[:, :])

            ones_f = pc.tile([1, 128], f32)
            nc.vector.memset(ones_f[:], 1.0)
            ones_r = pc.tile([1, 128], f32r)
            nc.vector.tensor_copy(ones_r[:], ones_f[:])
            bq_row_r = pc.tile([1, HD], f32r)
            nc.vector.tensor_copy(bq_row_r[:], bq_row[:])
            bqv_row_r = pc.tile([1, 130], f32r)
            nc.vector.tensor_copy(bqv_row_r[:], bqv_row[:])

            # bq as per-partition column (qhT/khT bias) and broadcast
            # across partitions (vh bias, with the 1.0 ones-columns)
            ps_t = pps.tile([128, 128], f32, tag="p1")
            nc.tensor.matmul(ps_t, bq_row_r[:], ones_r[:],
                             start=True, stop=True)
            bq_col = pc.tile([128, 1], f32)
            nc.vector.tensor_copy(bq_col[:], ps_t[:, 0:1])
            ps_t2 = pps.tile([128, 130], f32, tag="p1")
            nc.tensor.matmul(ps_t2, ones_r[:], bqv_row_r[:],
                             start=True, stop=True)
            bqv_bc = pc.tile([128, 130], f32)
            nc.vector.tensor_copy(bqv_bc[:], ps_t2[:])

            def dma_x(src, b, n):
                tiles = []
                for d in range(ND):
                    t = pin.tile([128, S], bf16, tag="xin", name=f"x{n}{b}{d}")
                    nc.sync.dma_start(t[:], src[d * 128:(d + 1) * 128,
                                                b * S:(b + 1) * S])
                    tiles.append(t)
                return tiles

            def proj_qk_fast(b):
                """d-major q,k projection on the sc-tag PSUM banks (free
                before attention starts); matmuls chase the DMA stream."""
                res = {}
                xt = {"q": dma_x(qT, b, "q"), "k": dma_x(kT, b, "k")}
                for name in ("q", "k"):
                    sb = pproj.tile([128, S], bf16, tag="proj" + name,
                                    name=f"proj{name}{b}")
                    pss = [pps.tile([128, HPC, 512], f32,
                                    tag="sc", name=f"pj{name}{b}{i}")
                           for i in range(2)]
                    for d in range(ND):
                        for j in range(NSC):
                            nc.tensor.matmul(pss[j // 2][:, j % 2, :],
                                             wq_sb[:, d, :],
                                             xt[name][d][:, j * 512:(j + 1) * 512],
                                             start=(d == 0), stop=(d == ND - 1))
                    for j in range(NSC):
                        nc.vector.tensor_scalar_add(
                            sb[:, j * 512:(j + 1) * 512],
                            pss[j // 2][:, j % 2, :], bq_col[:])
                    res[name] = sb
                return res["q"], res["k"]

            def qk_chain_thunks(hold, base):
                """b1 s-major projection as (min_iter, thunk) lane-A items."""
                items = []

                def dma_thunk():
                    hold["qt"] = dma_x(qT, 1, "q")
                    hold["kt"] = dma_x(kT, 1, "k")
                    hold["qh"] = pproj.tile([128, S], bf16, tag="projq",
                                            name="projq1")
                    hold["kh"] = pproj.tile([128, S], bf16, tag="projk",
                                            name="projk1")
                items.append((0, dma_thunk))
                for i, name in enumerate(("q", "k")):
                    for j in range(NSC):
                        def t1(name=name, j=j):
                            ps = pps.tile([128, 512], f32,
                                          tag="p1", name=f"pb{name}{j}")
                            xt = hold["qt" if name == "q" else "kt"]
                            for d in range(ND):
                                nc.tensor.matmul(ps, wq_sb[:, d, :],
                                                 xt[d][:, j * 512:(j + 1) * 512],
                                                 start=(d == 0),
                                                 stop=(d == ND - 1))
                            sb = hold["qh" if name == "q" else "kh"]
                            nc.vector.tensor_scalar_add(
                                sb[:, j * 512:(j + 1) * 512], ps, bq_col[:])
                        items.append((base[i] + 2 * j, t1))
                return items

            def dma_v(b, hold):
                hold["vt" + str(b)] = dma_x(vT, b, "v")
                hold["vh" + str(b)] = pvh.tile([128, NT, 130], bf16,
                                               tag="vh", name=f"vh{b}")

            def vh_items(b, hold, base, stride=1):
                items = []
                for t in range(NT):
                    def tt(t=t):
                        ps = pps.tile([128, 130], f32,
                                      tag="p1", name=f"pvh{t}")
                        vt = hold["vt" + str(b)]
                        for d in range(ND):
                            nc.tensor.matmul(ps, vt[d][:, t * 128:(t + 1) * 128],
                                             wqv_sb[:, d, :],
                                             start=(d == 0), stop=(d == ND - 1))
                        nc.vector.tensor_add(hold["vh" + str(b)][:, t, :],
                                             ps[:], bqv_bc[:])
                    items.append((base + stride * t, tt))
                return items

            # ---- two-lane deferred work queue ----
            laneA = []   # (min_iter, thunk): DMA-gated b1 projection work
            laneB = []   # always-ready deferred attention work
            it = [0]

            def pump():
                popped = 0
                if laneA and laneA[0][0] <= it[0]:
                    laneA.pop(0)[1]()
                    popped = 1
                for _ in range(2 - popped):
                    if laneB:
                        laneB.pop(0)()
                it[0] += 1

            def defer_attnv(b, sc, exs, vh_of, oT):
                ssl = slice(sc * 512, (sc + 1) * 512)
                cell = {}
                for t in range(NT):
                    def av(t=t):
                        if t == 0:
                            cell["att"] = [
                                pps.tile([65, 512], f32, tag="att",
                                         name=f"att{b}{sc}{h}")
                                for h in range(HPC)]
                        vh = vh_of()
                        for h in range(HPC):
                            nc.tensor.matmul(cell["att"][h],
                                             vh[:, t, h * 65:h * 65 + 65],
                                             exs[t][:, h, :],
                                             start=(t == 0), stop=(t == NT - 1))
                    laneB.append(av)

                def norm():
                    att = cell["att"]
                    asbs = []
                    for h in range(HPC):
                        asb = psmall.tile([65, 512], f32, tag="asb",
                                          bufs=4, name=f"asb{h}")
                        nc.vector.tensor_copy(asb[:], att[h][:])
                        asbs.append(asb)
                    for h in range(HPC):
                        den_r = psmall.tile([1, 512], f32r, tag="rec")
                        nc.vector.tensor_copy(den_r[:], asbs[h][64:65, :])
                        bcd = pps.tile([64, 512], f32, tag="p1")
                        nc.tensor.matmul(bcd, ones_r[:, 0:64], den_r[:],
                                         start=True, stop=True)
                        bcs = psmall.tile([64, 512], f32, tag="bcs")
                        nc.vector.reciprocal_approx_fast(bcs[:], bcd[:])
                        nc.vector.tensor_mul(oT[h * 64:(h + 1) * 64, ssl],
                                             asbs[h][0:64, :], bcs[:])
                laneB.append(norm)

                for g in range(2):
                    def op(g=g):
                        for u in range(4):
                            s1 = sc * 4 + (g * 2 + u // 2)
                            n = u % 2
                            s0 = s1 * 128
                            rs = slice(b * S + s0, b * S + s0 + 128)
                            nsl = slice(n * 512, (n + 1) * 512)
                            ps = pps.tile([128, 512], f32,
                                          tag="p1", name="oppsum")
                            nc.tensor.matmul(ps, oT[:, s0:s0 + 128],
                                             wo_sb[:, nsl],
                                             start=True, stop=True)
                            ob = pob.tile([128, 512], bf16, tag="ob")
                            nc.vector.tensor_copy(ob[:], ps)
                            nc.gpsimd.dma_start(out[rs, nsl], ob[:])
                    laneB.append(op)

            def inline_tail(b, sc, att, oT):
                ssl = slice(sc * 512, (sc + 1) * 512)
                asbs = []
                for h in range(HPC):
                    asb = psmall.tile([65, 512], f32, tag="asb",
                                      bufs=4, name=f"asbL{h}")
                    nc.vector.tensor_copy(asb[:], att[h][:])
                    asbs.append(asb)
                for h in range(HPC):
                    den_r = psmall.tile([1, 512], f32r, tag="rec")
                    nc.vector.tensor_copy(den_r[:], asbs[h][64:65, :])
                    bcd = pps.tile([64, 512], f32, tag="p1")
                    nc.tensor.matmul(bcd, ones_r[:, 0:64], den_r[:],
                                     start=True, stop=True)
                    bcs = psmall.tile([64, 512], f32, tag="bcs")
                    nc.vector.reciprocal_approx_fast(bcs[:], bcd[:])
                    nc.vector.tensor_mul(oT[h * 64:(h + 1) * 64, ssl],
                                         asbs[h][0:64, :], bcs[:])
                for s1 in range(4):
                    s0 = sc * 512 + s1 * 128
                    rs = slice(b * S + s0, b * S + s0 + 128)
                    for n in range(D // 512):
                        nsl = slice(n * 512, (n + 1) * 512)
                        ps = pps.tile([128, 512], f32, tag="p1",
                                      name="oppsumL")
                        nc.tensor.matmul(ps, oT[:, s0:s0 + 128],
                                         wo_sb[:, nsl], start=True, stop=True)
                        ob = pob.tile([128, 512], bf16, tag="ob")
                        nc.vector.tensor_copy(ob[:], ps)
                        nc.gpsimd.dma_start(out[rs, nsl], ob[:])

            def attention(b, qh, kh, vh_of, last=False):
                oT = poutT.tile([128, S], bf16, tag="outT", name=f"oT{b}")
                for sc in range(NSC):
                    inline = last and sc == NSC - 1
                    exs = []
                    att = None
                    for t in range(NT):
                        pump()
                        scps = pps.tile([128, HPC, 512], f32, tag="sc")
                        for h in range(HPC):
                            hp = slice(h * 64, (h + 1) * 64)
                            nc.tensor.matmul(scps[:, h, :],
                                             kh[hp, t * 128:(t + 1) * 128],
                                             qh[hp, sc * 512:(sc + 1) * 512],
                                             start=True, stop=True)
                        ex = pexp.tile([128, HPC, 512], bf16, tag="exp")
                        nc.scalar.activation(ex[:], scps[:], Exp, scale=0.125)
                        exs.append(ex)
                        if inline:
                            if att is None:
                                att = [pps.tile([65, 512], f32, tag="att",
                                                name=f"attL{h}")
                                       for h in range(HPC)]
                            vh = vh_of()
                            for h in range(HPC):
                                nc.tensor.matmul(att[h],
                                                 vh[:, t, h * 65:h * 65 + 65],
                                                 ex[:, h, :],
                                                 start=(t == 0),
                                                 stop=(t == NT - 1))
                    if inline:
                        inline_tail(b, sc, att, oT)
                    else:
                        defer_attnv(b, sc, exs, vh_of, oT)

            # ---- emission ----
            qh0, kh0 = proj_qk_fast(0)
            hold = {}
            dma_v(0, hold)
            # lane A thresholds are scores-iterations (~1.1us each from
            # ~28us): v0 lands ~36 -> iter 8; q1 ~48 -> 18; k1 ~60 -> 29;
            # v1 trigger at 20; v1 lands ~72 -> 40
            laneA.extend(vh_items(0, hold, 8))
            qk1 = qk_chain_thunks(hold, (18, 29))
            laneA.append(qk1[0])
            laneA.extend(qk1[1:])
            laneA.append((20, lambda: dma_v(1, hold)))
            laneA.extend(vh_items(1, hold, 40, stride=2))

            attention(0, qh0, kh0, lambda: hold["vh0"])
            attention(1, hold["qh"], hold["kh"], lambda: hold["vh1"],
                      last=True)
            while laneA or laneB:
                if laneA:
                    laneA.pop(0)[1]()
                if laneB:
                    laneB.pop(0)()

    nc.compile()
    return nc


def make_in_maps(q, k, v, Wq, bq, Wo):
    bf = ml_dtypes.bfloat16
    xT = {}
    for name, x in (("qT", q), ("kT", k), ("vT", v)):
        xT[name] = np.ascontiguousarray(
            np.asarray(x, np.float32).reshape(BS, D).T).astype(bf)

    in_maps = []
    for c in range(NCORES):
        cols = slice(c * HD, (c + 1) * HD)
        wqc = np.asarray(Wq, np.float32)[:, cols]
        bqc = np.asarray(bq, np.float32)[cols]
        wqve = np.zeros((D, 130), np.float32)
        wqve[:, 0:64] = wqc[:, 0:64]
        wqve[:, 65:129] = wqc[:, 64:128]
        bqve = np.zeros((1, 130), np.float32)
        bqve[0, 0:64] = bqc[0:64]
        bqve[0, 65:129] = bqc[64:128]
        bqve[0, 64] = 1.0
        bqve[0, 129] = 1.0
        in_maps.append({
            "qT": xT["qT"], "kT": xT["kT"], "vT": xT["vT"],
            "wq": np.ascontiguousarray(wqc).astype(bf),
            "wqv": wqve.astype(bf),
            "bq": bqc[None, :].copy(),
            "bqv": bqve,
            "wo": np.ascontiguousarray(np.asarray(Wo, np.float32)[cols, :]).astype(bf),
        })
    return in_maps


def kernel(q, k, v, Wq, bq, Wo, bo):
    from concourse.bass_utils import run_bass_kernel_spmd

    if "nc" not in _cache:
        _cache["nc"] = _build()
    nc = _cache["nc"]

    in_maps = make_in_maps(q, k, v, Wq, bq, Wo)
    res = run_bass_kernel_spmd(nc, in_maps, list(range(NCORES)), trace=False)
    acc = np.zeros((BS, D), np.float64)
    for c in range(NCORES):
        acc += res.results[c]["out"].astype(np.float64)
    acc += np.asarray(bo, np.float32)[None, :].astype(np.float64)
    return acc.reshape(B, S, D).astype(np.float32)


# revision 29
# speedup vs baseline: 1.0315x; 1.0315x over previous
"""Multi-head attention (B=2, S=2048, D=1024, H=16, d_k=64) on 8 TRN2 NeuronCores.

Sharding: head-parallel. Core c owns heads (2c, 2c+1) for both batch rows:
 - replicated inputs: qT/kT/vT = x.reshape(B*S, D).T  in bf16, [1024, 4096]
   (D on partitions so the TensorEngine contracts over D with no transposes)
 - per-core weights: Wq columns / Wo rows for its two heads
 - per-core output: partial = attn_out(own heads) @ Wo[own rows]  [4096, 1024] bf16
   The host sums the 8 partials (f32) and adds bo.  No cross-core comm.

Per-core dataflow (bf16 matmuls, f32 PSUM):
 1. qhT/khT [128, 2048] per batch = Wq_c.T @ xT (+bq), d-major accumulation.
    vh [2048, 130] natural = vT.T @ Wqv_c; Wqv has zero-cols / bqv has
    1.0-cols so each head gets a ones column -> attn@V also produces the
    softmax denominators.
 2. scoresT[t,s] = khT.T @ qhT, both heads packed into disjoint PE
    row-groups (K=64).  exp(x/8) on ScalarE from PSUM, bf16 out.
 3. attn@V accumulated over t; row 64 = denominator.  Normalize via
    f32r denominator broadcast (K=1 matmul) + fast reciprocal + mul.
 4. partial[s, :] = outT.T @ Wo_c -> bf16 -> DRAM.

Scheduling: ScalarE (exp, ~142us) and the TensorEngine (~150us) are both
near-saturated, so emission order keeps ACT fed: each s-chunk's
scores+exp loop is emitted first; its attn@V/normalize/out-proj are
deferred one s-chunk and re-emitted between later score iterations via a
two-lane work queue (lane A: DMA-gated b1 projection work with
earliest-iteration thresholds; lane B: always-ready deferred attention).
"""

import numpy as np
import ml_dtypes

B, S, D, H, DK = 2, 2048, 1024, 16, 64
NCORES = 8
HPC = H // NCORES          # heads per core = 2
BS = B * S                 # 4096
HD = HPC * DK              # 128 = per-core head dims

_cache = {}


def _build():
    import concourse.bass as bass
    import concourse.tile as tile
    from concourse import bacc, mybir

    f32 = mybir.dt.float32
    f32r = mybir.dt.float32r
    bf16 = mybir.dt.bfloat16
    Exp = mybir.ActivationFunctionType.Exp

    nc = bacc.Bacc("TRN2", target_bir_lowering=False, debug=False,
                   num_devices=NCORES)

    qT = nc.declare_dram_parameter("qT", [D, BS], bf16, isOutput=False)
    kT = nc.declare_dram_parameter("kT", [D, BS], bf16, isOutput=False)
    vT = nc.declare_dram_parameter("vT", [D, BS], bf16, isOutput=False)
    wq = nc.declare_dram_parameter("wq", [D, HD], bf16, isOutput=False)
    wqv = nc.declare_dram_parameter("wqv", [D, 130], bf16, isOutput=False)
    bq = nc.declare_dram_parameter("bq", [1, HD], f32, isOutput=False)
    bqv = nc.declare_dram_parameter("bqv", [1, 130], f32, isOutput=False)
    wo = nc.declare_dram_parameter("wo", [HD, D], bf16, isOutput=False)
    out = nc.declare_dram_parameter("out", [BS, D], bf16, isOutput=True)

    ND = D // 128            # 8 d-chunks
    NT = S // 128            # 16 t-chunks per batch
    NSC = S // 512           # 4 s-chunks per batch

    with tile.TileContext(nc) as tc:
        with (
            tc.tile_pool(name="const", bufs=1) as pc,
            tc.tile_pool(name="xin", bufs=24) as pin,
            tc.tile_pool(name="proj", bufs=2) as pproj,
            tc.tile_pool(name="vh", bufs=2) as pvh,
            tc.tile_pool(name="exp", bufs=20) as pexp,
            tc.tile_pool(name="outT", bufs=2) as poutT,
            tc.tile_pool(name="small", bufs=2) as psmall,
            tc.tile_pool(name="ob", bufs=4) as pob,
            tc.tile_pool(name="ps", bufs=2, space="PSUM") as pps,
        ):
            # ---- constants (gpsimd queue; bq first — PE needs it early) ----
            bq_row = pc.tile([1, HD], f32)
            nc.gpsimd.dma_start(bq_row[:], bq[:, :])
            bqv_row = pc.tile([1, 130], f32)
            nc.gpsimd.dma_start(bqv_row[:], bqv[:, :])
            wq_sb = pc.tile([128, ND, HD], bf16)
            for d in range(ND):
                nc.gpsimd.dma_start(wq_sb[:, d, :], wq[d * 128:(d + 1) * 128, :])
            wqv_sb = pc.tile([128, ND, 130], bf16)
            for d in range(ND):
                nc.gpsimd.dma_start(wqv_sb[:, d, :], wqv[d * 128:(d + 1) * 128, :])
            wo_sb = pc.tile([HD, D], bf16)
            nc.gpsimd.dma_start(wo_sb[:], wo[:, :])

            ones_f = pc.tile([1, 128], f32)
            nc.vector.memset(ones_f[:], 1.0)
            ones_r = pc.tile([1, 128], f32r)
            nc.vector.tensor_copy(ones_r[:], ones_f[:])
            bq_row_r = pc.tile([1, HD], f32r)
            nc.vector.tensor_copy(bq_row_r[:], bq_row[:])
            bqv_row_r = pc.tile([1, 130], f32r)
            nc.vector.tensor_copy(bqv_row_r[:], bqv_row[:])

            # bq as per-partition column (qhT/khT bias) and broadcast
            # across partitions (vh bias, with the 1.0 ones-columns)
            ps_t = pps.tile([128, 128], f32, tag="p1")
            nc.tensor.matmul(ps_t, bq_row_r[:], ones_r[:],
                             start=True, stop=True)
            bq_col = pc.tile([128, 1], f32)
            nc.vector.tensor_copy(bq_col[:], ps_t[:, 0:1])
            ps_t2 = pps.tile([128, 130], f32, tag="p1")
            nc.tensor.matmul(ps_t2, ones_r[:], bqv_row_r[:],
                             start=True, stop=True)
            bqv_bc = pc.tile([128, 130], f32)
            nc.vector.tensor_copy(bqv_bc[:], ps_t2[:])

            def dma_x(src, b, n):
                tiles = []
                for d in range(ND):
                    t = pin.tile([128, S], bf16, tag="xin", name=f"x{n}{b}{d}")
                    nc.sync.dma_start(t[:], src[d * 128:(d + 1) * 128,
                                                b * S:(b + 1) * S])
                    tiles.append(t)
                return tiles

            def proj_qk_fast(b):
                """d-major q,k projection on the sc-tag PSUM banks (free
                before attention starts); matmuls chase the DMA stream."""
                res = {}
                xt = {"q": dma_x(qT, b, "q"), "k": dma_x(kT, b, "k")}
                for name in ("q", "k"):
                    sb = pproj.tile([128, S], bf16, tag="proj" + name,
                                    name=f"proj{name}{b}")
                    pss = [pps.tile([128, HPC, 512], f32,
                                    tag="sc", name=f"pj{name}{b}{i}")
                           for i in range(2)]
                    for d in range(ND):
                        for j in range(NSC):
                            nc.tensor.matmul(pss[j // 2][:, j % 2, :],
                                             wq_sb[:, d, :],
                                             xt[name][d][:, j * 512:(j + 1) * 512],
                                             start=(d == 0), stop=(d == ND - 1))
                    for j in range(NSC):
                        nc.vector.tensor_scalar_add(
                            sb[:, j * 512:(j + 1) * 512],
                            pss[j // 2][:, j % 2, :], bq_col[:])
                    res[name] = sb
                return res["q"], res["k"]

            def qk_chain_thunks(hold, base):
                """b1 s-major projection as (min_iter, thunk) lane-A items."""
                items = []

                def dma_thunk():
                    hold["qt"] = dma_x(qT, 1, "q")
                    hold["kt"] = dma_x(kT, 1, "k")
                    hold["qh"] = pproj.tile([128, S], bf16, tag="projq",
                                            name="projq1")
                    hold["kh"] = pproj.tile([128, S], bf16, tag="projk",
                                            name="projk1")
                items.append((0, dma_thunk))
                for i, name in enumerate(("q", "k")):
                    for j in range(NSC):
                        def t1(name=name, j=j):
                            ps = pps.tile([128, 512], f32,
                                          tag="p1", name=f"pb{name}{j}")
                            xt = hold["qt" if name == "q" else "kt"]
                            for d in range(ND):
                                nc.tensor.matmul(ps, wq_sb[:, d, :],
                                                 xt[d][:, j * 512:(j + 1) * 512],
                                                 start=(d == 0),
                                                 stop=(d == ND - 1))
                            sb = hold["qh" if name == "q" else "kh"]
                            nc.vector.tensor_scalar_add(
                                sb[:, j * 512:(j + 1) * 512], ps, bq_col[:])
                        items.append((base[i] + j, t1))
                return items

            def dma_v(b, hold):
                hold["vt" + str(b)] = dma_x(vT, b, "v")
                hold["vh" + str(b)] = pvh.tile([128, NT, 130], bf16,
                                               tag="vh", name=f"vh{b}")

            def vh_items(b, hold, base, stride=1):
                items = []
                for t in range(NT):
                    def tt(t=t):
                        ps = pps.tile([128, 130], f32,
                                      tag="p1", name=f"pvh{t}")
                        vt = hold["vt" + str(b)]
                        for d in range(ND):
                            nc.tensor.matmul(ps, vt[d][:, t * 128:(t + 1) * 128],
                                             wqv_sb[:, d, :],
                                             start=(d == 0), stop=(d == ND - 1))
                        nc.vector.tensor_add(hold["vh" + str(b)][:, t, :],
                                             ps[:], bqv_bc[:])
                    items.append((base + stride * t, tt))
                return items

            # ---- two-lane deferred work queue ----
            laneA = []   # (min_iter, thunk): DMA-gated b1 projection work
            laneB = []   # always-ready deferred attention work
            it = [0]

            def pump():
                popped = 0
                if laneA and laneA[0][0] <= it[0]:
                    laneA.pop(0)[1]()
                    popped = 1
                for _ in range(2 - popped):
                    if laneB:
                        laneB.pop(0)()
                it[0] += 1

            def defer_attnv(b, sc, exs, vh_of, oT):
                ssl = slice(sc * 512, (sc + 1) * 512)
                cell = {}
                for t in range(NT):
                    def av(t=t):
                        if t == 0:
                            cell["att"] = [
                                pps.tile([65, 512], f32, tag="att",
                                         name=f"att{b}{sc}{h}")
                                for h in range(HPC)]
                        vh = vh_of()
                        for h in range(HPC):
                            nc.tensor.matmul(cell["att"][h],
                                             vh[:, t, h * 65:h * 65 + 65],
                                             exs[t][:, h, :],
                                             start=(t == 0), stop=(t == NT - 1))
                    laneB.append(av)

                def norm():
                    att = cell["att"]
                    asbs = []
                    for h in range(HPC):
                        asb = psmall.tile([65, 512], f32, tag="asb",
                                          bufs=4, name=f"asb{h}")
                        nc.vector.tensor_copy(asb[:], att[h][:])
                        asbs.append(asb)
                    for h in range(HPC):
                        den_r = psmall.tile([1, 512], f32r, tag="rec")
                        nc.vector.tensor_copy(den_r[:], asbs[h][64:65, :])
                        bcd = pps.tile([64, 512], f32, tag="p1")
                        nc.tensor.matmul(bcd, ones_r[:, 0:64], den_r[:],
                                         start=True, stop=True)
                        bcs = psmall.tile([64, 512], f32, tag="bcs")
                        nc.vector.reciprocal_approx_fast(bcs[:], bcd[:])
                        nc.vector.tensor_mul(oT[h * 64:(h + 1) * 64, ssl],
                                             asbs[h][0:64, :], bcs[:])
                laneB.append(norm)

                for g in range(2):
                    def op(g=g):
                        for u in range(4):
                            s1 = sc * 4 + (g * 2 + u // 2)
                            n = u % 2
                            s0 = s1 * 128
                            rs = slice(b * S + s0, b * S + s0 + 128)
                            nsl = slice(n * 512, (n + 1) * 512)
                            ps = pps.tile([128, 512], f32,
                                          tag="p1", name="oppsum")
                            nc.tensor.matmul(ps, oT[:, s0:s0 + 128],
                                             wo_sb[:, nsl],
                                             start=True, stop=True)
                            ob = pob.tile([128, 512], bf16, tag="ob")
                            nc.vector.tensor_copy(ob[:], ps)
                            nc.gpsimd.dma_start(out[rs, nsl], ob[:])
                    laneB.append(op)

            def inline_tail(b, sc, att, oT):
                ssl = slice(sc * 512, (sc + 1) * 512)
                asbs = []
                for h in range(HPC):
                    asb = psmall.tile([65, 512], f32, tag="asb",
                                      bufs=4, name=f"asbL{h}")
                    nc.vector.tensor_copy(asb[:], att[h][:])
                    asbs.append(asb)
                for h in range(HPC):
                    den_r = psmall.tile([1, 512], f32r, tag="rec")
                    nc.vector.tensor_copy(den_r[:], asbs[h][64:65, :])
                    bcd = pps.tile([64, 512], f32, tag="p1")
                    nc.tensor.matmul(bcd, ones_r[:, 0:64], den_r[:],
                                     start=True, stop=True)
                    bcs = psmall.tile([64, 512], f32, tag="bcs")
                    nc.vector.reciprocal_approx_fast(bcs[:], bcd[:])
                    nc.vector.tensor_mul(oT[h * 64:(h + 1) * 64, ssl],
                                         asbs[h][0:64, :], bcs[:])
                for s1 in range(4):
                    s0 = sc * 512 + s1 * 128
                    rs = slice(b * S + s0, b * S + s0 + 128)
                    for n in range(D // 512):
                        nsl = slice(n * 512, (n + 1) * 512)
                        ps = pps.tile([128, 512], f32, tag="p1",
                                      name="oppsumL")
                        nc.tensor.matmul(ps, oT[:, s0:s0 + 128],
                                         wo_sb[:, nsl], start=True, stop=True)
                        ob = pob.tile([128, 512], bf16, tag="ob")
                        nc.vector.tensor_copy(ob[:], ps)
                        nc.gpsimd.dma_start(out[rs, nsl], ob[:])

            def attention(b, qh, kh, vh_of, last=False):
                oT = poutT.tile([128, S], bf16, tag="outT", name=f"oT{b}")
                for sc in range(NSC):
                    inline = last and sc == NSC - 1
                    exs = []
                    att = None
                    for t in range(NT):
                        pump()
                        scps = pps.tile([128, HPC, 512], f32, tag="sc")
                        for h in range(HPC):
                            hp = slice(h * 64, (h + 1) * 64)
                            nc.tensor.matmul(scps[:, h, :],
                                             kh[hp, t * 128:(t + 1) * 128],
                                             qh[hp, sc * 512:(sc + 1) * 512],
                                             start=True, stop=True)
                        ex = pexp.tile([128, HPC, 512], bf16, tag="exp")
                        nc.scalar.activation(ex[:], scps[:], Exp, scale=0.125)
                        exs.append(ex)
                        if inline:
                            if att is None:
                                att = [pps.tile([65, 512], f32, tag="att",
                                                name=f"attL{h}")
                                       for h in range(HPC)]
                            vh = vh_of()
                            for h in range(HPC):
                                nc.tensor.matmul(att[h],
                                                 vh[:, t, h * 65:h * 65 + 65],
                                                 ex[:, h, :],
                                                 start=(t == 0),
                                                 stop=(t == NT - 1))
                    if inline:
                        inline_tail(b, sc, att, oT)
                    else:
                        defer_attnv(b, sc, exs, vh_of, oT)

            # ---- emission ----
            qh0, kh0 = proj_qk_fast(0)
            hold = {}
            dma_v(0, hold)
            # lane A thresholds are scores-iterations (~1.1us each from
            # ~28us): v0 lands ~36 -> iter 8; q1 ~48 -> 18; k1 ~60 -> 29;
            # v1 trigger at 20; v1 lands ~72 -> 40
            laneA.extend(vh_items(0, hold, 8))
            qk1 = qk_chain_thunks(hold, (18, 29))
            laneA.append(qk1[0])
            laneA.extend(qk1[1:])
            laneA.append((20, lambda: dma_v(1, hold)))
            laneA.extend(vh_items(1, hold, 40))

            attention(0, qh0, kh0, lambda: hold["vh0"])
            attention(1, hold["qh"], hold["kh"], lambda: hold["vh1"])
            wps = pps.tile([128, 512], f32, tag="p1", name="warmtail")

            def warm(n):
                for _ in range(n):
                    nc.tensor.matmul(wps, wq_sb[:, 0, :], wq_sb[:, 0:4, :],
                                     start=True, stop=True,
                                     skip_group_check=True)

            while laneA or laneB:
                if laneA:
                    laneA.pop(0)[1]()
                if laneB:
                    th = laneB.pop(0)()
                    if len(laneB) == 3:   # after last attnV, before norm
                        warm(8)
                    elif len(laneB) == 2:  # after norm, before outproj
                        warm(8)

    nc.compile()
    return nc


def make_in_maps(q, k, v, Wq, bq, Wo):
    bf = ml_dtypes.bfloat16
    xT = {}
    for name, x in (("qT", q), ("kT", k), ("vT", v)):
        xT[name] = np.ascontiguousarray(
            np.asarray(x, np.float32).reshape(BS, D).T).astype(bf)

    in_maps = []
    for c in range(NCORES):
        cols = slice(c * HD, (c + 1) * HD)
        wqc = np.asarray(Wq, np.float32)[:, cols]
        bqc = np.asarray(bq, np.float32)[cols]
        wqve = np.zeros((D, 130), np.float32)
        wqve[:, 0:64] = wqc[:, 0:64]
        wqve[:, 65:129] = wqc[:, 64:128]
        bqve = np.zeros((1, 130), np.float32)
        bqve[0, 0:64] = bqc[0:64]
        bqve[0, 65:129] = bqc[64:128]
        bqve[0, 64] = 1.0
        bqve[0, 129] = 1.0
        in_maps.append({
            "qT": xT["qT"], "kT": xT["kT"], "vT": xT["vT"],
            "wq": np.ascontiguousarray(wqc).astype(bf),
            "wqv": wqve.astype(bf),
            "bq": bqc[None, :].copy(),
            "bqv": bqve,
            "wo": np.ascontiguousarray(np.asarray(Wo, np.float32)[cols, :]).astype(bf),
        })
    return in_maps


def kernel(q, k, v, Wq, bq, Wo, bo):
    from concourse.bass_utils import run_bass_kernel_spmd

    if "nc" not in _cache:
        _cache["nc"] = _build()
    nc = _cache["nc"]

    in_maps = make_in_maps(q, k, v, Wq, bq, Wo)
    res = run_bass_kernel_spmd(nc, in_maps, list(range(NCORES)), trace=False)
    acc = np.zeros((BS, D), np.float64)
    for c in range(NCORES):
        acc += res.results[c]["out"].astype(np.float64)
    acc += np.asarray(bo, np.float32)[None, :].astype(np.float64)
    return acc.reshape(B, S, D).astype(np.float32)
